# revision 1
# baseline (speedup 1.0000x reference)
"""BrokenBiasAttention Trainium2 kernel (8-core SPMD).

Sharding: core c -> batch b=c//2, query-row-half r=c%2 (1024 of 2048 rows).
Each core computes q for its rows, k/v for the whole batch, full 8-head
attention for its rows, and the output projection for its rows. Outputs are
disjoint row blocks -> gather is pure concatenation.

Device algorithm (per core):
  - all matmuls in bf16 (weights/x cast on host)
  - scores^T tiles [krow 128, qrow 512] via row-packed K=32 matmuls
  - softmax without max-subtraction (scores bounded ~|10|), constant shift 20:
      attn_un = exp(s - 20) * expF,   expF = exp(bias) gathered on device
  - bias is 3-level block-Toeplitz: host stages TW[h,rdw,w2,rh,w1] =
      T[h, 4r+rdw, rh, 15+w1-w2]  (pure replication / layout staging);
    device exps it once (small) and DMA-gathers 256-elem contiguous runs to
    build expF[h, rdw, half][128, 256] tiles in SBUF.
  - attn@v + rowsum via column-tiled matmuls accumulating in one PSUM bank
  - normalize: one DVE reciprocal per epilogue + DRAM-bounce broadcast
  - bias-multiply split between DVE and GpSimd.
"""

import math
import sys

import numpy as np

if "/opt/trn_rl_repo" not in sys.path:
    sys.path.insert(0, "/opt/trn_rl_repo")

N = 2048
C = 256
NH = 8
HD = 32
B = 4
QR = 1024  # q rows per core
S_SHIFT = 20.0

_NC = None


def _build_nc(dbg=False):
    import concourse.bass as bass
    import concourse.tile as tile
    from concourse import bacc, mybir
    from concourse.bass import ds, ts

    f32 = mybir.dt.float32
    bf16 = mybir.dt.bfloat16
    EXP = mybir.ActivationFunctionType.Exp

    nc = bacc.Bacc(None, target_bir_lowering=False, debug=False)

    xT = nc.dram_tensor("xT", [C, N], bf16, kind="ExternalInput")
    xTq = nc.dram_tensor("xTq", [C, QR], bf16, kind="ExternalInput")
    Wq_d = nc.dram_tensor("Wq", [C, C], bf16, kind="ExternalInput")
    Wk_d = nc.dram_tensor("Wk", [C, C], bf16, kind="ExternalInput")
    Wv_d = nc.dram_tensor("Wv", [C, C], bf16, kind="ExternalInput")
    Wo_d = nc.dram_tensor("Wo", [C, C], bf16, kind="ExternalInput")
    # TW[h, rdw(11), w2(16), rh(31), w1(16)]
    TW_d = nc.dram_tensor("TW", [NH, 11, 16, 31, 16], f32, kind="ExternalInput")
    out_d = nc.dram_tensor("out", [QR, C], f32, kind="ExternalOutput")

    assert 2 * 11 * 16 * 31 * 16 == 128 * 1364

    with tile.TileContext(nc) as tc:
        with (
            tc.tile_pool(name="consts", bufs=1) as consts,
            tc.tile_pool(name="twp", bufs=2) as twp,
            tc.tile_pool(name="etwp", bufs=2) as etwp,
            tc.tile_pool(name="expfp", bufs=1) as expfp,
            tc.tile_pool(name="xp", bufs=3) as xp,
            tc.tile_pool(name="kqv", bufs=1) as kqv,
            tc.tile_pool(name="ep", bufs=6) as ep,
            tc.tile_pool(name="rp", bufs=2) as rp,
            tc.tile_pool(name="otp", bufs=2) as otp,
            tc.tile_pool(name="stp", bufs=2) as stp,
            tc.tile_pool(name="spsum", bufs=3, space="PSUM") as spsum,
            tc.tile_pool(name="apsum", bufs=2, space="PSUM") as apsum,
            tc.tile_pool(name="dramp", bufs=4, space="DRAM") as dramp,
        ):
            # ---- expF construction: TW -> exp -> dram -> gather ----
            expf_sb = expfp.tile([128, NH * 11 * 384], bf16, tag="expf")
            expf_view = expf_sb.rearrange(
                "p (h r f) -> p h r f", h=NH, r=11, f=384
            )
            etw_d = dramp.tile([4, 128, 1364], bf16, name="etw_d")
            for hp in range(4):
                tw_sb = twp.tile([128, 1364], f32, tag="tw")
                src = TW_d[ds(2 * hp, 2)].rearrange(
                    "h r w2 rh w1 -> (h r w2 rh w1)"
                ).rearrange("(p f) -> p f", p=128)
                nc.scalar.dma_start(out=tw_sb, in_=src)
                etw_sb = etwp.tile([128, 1364], bf16, tag="etw")
                nc.scalar.activation(etw_sb, tw_sb, EXP)
                nc.scalar.dma_start(out=etw_d[hp], in_=etw_sb)
                # gather per h2': dest 16 partitions, free (2*rdw 22, 384)
                # union rh window rows 7-h2' .. 31-h2' (24 rows) covers both halves
                for h2p in range(8):
                    gap = bass.AP(
                        tensor=etw_d.tensor,
                        offset=etw_d.offset + hp * 174592 + (7 - h2p) * 16,
                        ap=[
                            [496, 16],    # w2 (partition)
                            [7936, 22],   # (h in pair, rdw) merged
                            [1, 384],     # (rh-window, w1) contiguous run
                        ],
                    )
                    geng = nc.gpsimd if h2p % 2 == 0 else nc.sync
                    geng.dma_start(
                        out=expf_view[ds(16 * h2p, 16), ds(2 * hp, 2)], in_=gap
                    )

            # ---- constants ----
            w_sb = {}
            for name, d in (("Wq", Wq_d), ("Wk", Wk_d), ("Wv", Wv_d), ("Wo", Wo_d)):
                t = consts.tile([128, 2, C], bf16, tag=f"w_{name}", name=f"w_{name}")
                nc.sync.dma_start(out=t, in_=d[:].rearrange("(ch p) n -> p ch n", p=128))
                w_sb[name] = t
            ones_sb = consts.tile([128, 32], bf16, tag="ones")
            nc.vector.memset(ones_sb, 1.0)
            ebias = consts.tile([128, 1], f32, tag="ebias")
            nc.vector.memset(ebias, -S_SHIFT)

            if dbg:
                dbg_expf = nc.dram_tensor(
                    "dbg_expf", [128, NH * 11 * 384], bf16,
                    kind="ExternalOutput")
                nc.sync.dma_start(out=dbg_expf[:], in_=expf_sb)

            # ---- projections (all bf16) ----
            kT_sb = [kqv.tile([128, N], bf16, tag=f"kT{m}", name=f"kT{m}")
                     for m in range(2)]
            qT_sb = [kqv.tile([128, QR], bf16, tag=f"qT{m}", name=f"qT{m}")
                     for m in range(2)]
            v_sb = kqv.tile([128, 16, C], bf16, tag="v")
            qscale = 1.0 / math.sqrt(HD)

            xTq_r = xTq[:].rearrange("(ch p) n -> p ch n", p=128)
            for j in range(QR // 512):
                xq = xp.tile([128, 2, 512], bf16, tag="x")
                nc.sync.dma_start(out=xq, in_=xTq_r[:, :, ds(512 * j, 512)])
                for m in range(2):
                    ps = spsum.tile([128, 1024], f32, tag="s")
                    for ch in range(2):
                        nc.tensor.matmul(
                            ps[:, :512],
                            lhsT=w_sb["Wq"][:, ch, ts(m, 128)],
                            rhs=xq[:, ch, :],
                            start=(ch == 0),
                            stop=(ch == 1),
                        )
                    nc.vector.tensor_scalar_mul(
                        qT_sb[m][:, ds(512 * j, 512)], ps[:, :512], qscale
                    )

            xT_r = xT[:].rearrange("(ch p) n -> p ch n", p=128)
            for j in range(N // 512):
                xc = xp.tile([128, 2, 512], bf16, tag="x")
                nc.sync.dma_start(out=xc, in_=xT_r[:, :, ds(512 * j, 512)])
                for m in range(2):
                    ps = spsum.tile([128, 1024], f32, tag="s")
                    for ch in range(2):
                        nc.tensor.matmul(
                            ps[:, :512],
                            lhsT=w_sb["Wk"][:, ch, ts(m, 128)],
                            rhs=xc[:, ch, :],
                            start=(ch == 0),
                            stop=(ch == 1),
                        )
                    nc.vector.tensor_copy(kT_sb[m][:, ds(512 * j, 512)], ps[:, :512])
                for t in range(4):
                    kt = 4 * j + t
                    ps = spsum.tile([128, 1024], f32, tag="s")
                    for ch in range(2):
                        nc.tensor.matmul(
                            ps[:, :C],
                            lhsT=xc[:, ch, ts(t, 128)],
                            rhs=w_sb["Wv"][:, ch, :],
                            start=(ch == 0),
                            stop=(ch == 1),
                        )
                    nc.vector.tensor_copy(v_sb[:, kt, :], ps[:, :C])

            # ---- main attention loops ----
            oT_tiles = []
            for qc in range(2):
                oT = otp.tile([128, 2, 512], bf16, tag="oT", name=f"oT{qc}")
                oT_tiles.append(oT)
            for g2 in range(4):
                for qc in range(2):
                    oT = oT_tiles[qc]
                    po_av = 0 if g2 % 2 == 0 else 64
                    po_rs = 64 - po_av
                    half_idx = g2 // 2
                    acc = apsum.tile([128, 512], f32, tag="acc")
                    e_tiles = {}

                    def emit_av(kt):
                        e_t = e_tiles.pop(kt)
                        for k in range(2):
                            h = 2 * g2 + k
                            nc.tensor.matmul(
                                acc[ds(po_av + 32 * k, 32), :],
                                lhsT=v_sb[:, kt, ds(32 * h, 32)],
                                rhs=e_t[:, ts(k, 512)],
                                start=(kt == 0),
                                stop=(kt == 15),
                                tile_position=(0, po_av + 32 * k),
                                skip_group_check=True,
                            )
                            nc.tensor.matmul(
                                acc[ds(po_rs + 32 * k, 32), :],
                                lhsT=ones_sb,
                                rhs=e_t[:, ts(k, 512)],
                                start=(kt == 0),
                                stop=(kt == 15),
                                tile_position=(0, po_rs + 32 * k),
                                skip_group_check=True,
                            )

                    for kt in range(16):
                        s_ps = spsum.tile([128, 1024], f32, tag="s")
                        for k in range(2):
                            h = 2 * g2 + k
                            i = h % 4
                            nc.tensor.matmul(
                                s_ps[:, ts(k, 512)],
                                lhsT=kT_sb[half_idx][ds(32 * i, 32), ts(kt, 128)],
                                rhs=qT_sb[half_idx][ds(32 * i, 32), ts(qc, 512)],
                                start=True,
                                stop=True,
                                tile_position=(32 * i, 0),
                            )
                        e_sb = ep.tile([128, 1024], bf16, tag="e")
                        e_tiles[kt] = e_sb
                        nc.scalar.activation(e_sb, s_ps, EXP, bias=ebias[:, :])
                        rdw0 = 2 * qc - (kt // 2) + 7
                        woff = 128 if kt % 2 == 0 else 0
                        ev = e_sb.rearrange("p (k jj f) -> p k jj f", k=2, jj=2)
                        fv = expf_view[
                            :, ds(2 * g2, 2), ds(rdw0, 2), ds(woff, 256)
                        ]
                        nc.vector.tensor_mul(ev, ev, fv)
                        if kt >= 2:
                            emit_av(kt - 2)
                    emit_av(14)
                    emit_av(15)
                    # epilogue: normalize 2 heads into oT
                    recip = rp.tile([128, 512], f32, tag="recip")
                    rep = rp.tile([128, 512], f32, tag="rep")
                    nc.vector.tensor_copy(
                        recip[ds(po_rs, 64), :], acc[ds(po_rs, 64), :]
                    )
                    nc.vector.reciprocal(
                        recip[ds(po_rs, 64), :], recip[ds(po_rs, 64), :]
                    )
                    nc.sync.dma_start(
                        out=rep[ds(po_av, 64), :], in_=recip[ds(po_rs, 64), :]
                    )
                    nc.vector.tensor_mul(
                        oT[ds(po_av, 64), half_idx, :],
                        acc[ds(po_av, 64), :],
                        rep[ds(po_av, 64), :],
                    )
            # final projections (after both qc loops; off the loop critical path)
            for qc in range(2):
                oT = oT_tiles[qc]
                for s in range(4):
                    fps = spsum.tile([128, 1024], f32, tag="s")
                    for ch in range(2):
                        nc.tensor.matmul(
                            fps[:, :C],
                            lhsT=oT[:, ch, ts(s, 128)],
                            rhs=w_sb["Wo"][:, ch, :],
                            start=(ch == 0),
                            stop=(ch == 1),
                        )
                    stage = stp.tile([128, C], f32, tag="stage")
                    nc.vector.tensor_copy(stage, fps[:, :C])
                    nc.sync.dma_start(
                        out=out_d[ds(512 * qc + 128 * s, 128), :], in_=stage
                    )

    nc.compile()
    return nc


def _host_inputs(x, Wq, Wk, Wv, Wo, bias_table):
    """Build the 8 per-core input maps."""
    import ml_dtypes

    bf = ml_dtypes.bfloat16
    x = np.asarray(x, dtype=np.float32)
    T = np.asarray(bias_table, dtype=np.float32)
    xf = np.ascontiguousarray(x.reshape(B, N, C))
    idx_w = 15 + np.arange(16)[None, :] - np.arange(16)[:, None]  # [w2, w1]
    Ws = {
        "Wq": np.ascontiguousarray(np.asarray(Wq, np.float32).astype(bf)),
        "Wk": np.ascontiguousarray(np.asarray(Wk, np.float32).astype(bf)),
        "Wv": np.ascontiguousarray(np.asarray(Wv, np.float32).astype(bf)),
        "Wo": np.ascontiguousarray(np.asarray(Wo, np.float32).astype(bf)),
    }
    in_maps = []
    for c in range(8):
        b, r = c // 2, c % 2
        d1min = 4 * r
        Twin = T[:, d1min:d1min + 11]                     # [8, 11, 31, 31]
        TW = Twin[:, :, :, idx_w]                         # [8,11,31,16,16] (h,rdw,rh,w2,w1)
        TW = np.ascontiguousarray(TW.transpose(0, 1, 3, 2, 4))  # [h,rdw,w2,rh,w1]
        in_maps.append({
            "xT": np.ascontiguousarray(xf[b].T.astype(bf)),
            "xTq": np.ascontiguousarray(xf[b, QR * r:QR * (r + 1)].T.astype(bf)),
            "TW": TW,
            **Ws,
        })
    return in_maps


def kernel(x, Wq, Wk, Wv, Wo, bias_table, _results_hook=None):
    global _NC
    if _NC is None:
        _NC = _build_nc()
    from concourse.bass_utils import run_bass_kernel_spmd

    in_maps = _host_inputs(x, Wq, Wk, Wv, Wo, bias_table)
    res = run_bass_kernel_spmd(_NC, in_maps, core_ids=list(range(8)))
    if _results_hook is not None:
        _results_hook(res)
    out = np.zeros((B, N, C), dtype=np.float32)
    for c in range(8):
        b, r = c // 2, c % 2
        out[b, QR * r:QR * (r + 1)] = res.results[c]["out"]
    D, H, W = 8, 16, 16
    return out.reshape(B, D, H, W, C)



# revision 14
# speedup vs baseline: 1.1179x; 1.1179x over previous
"""BrokenBiasAttention Trainium2 kernel (8-core SPMD), v2.

Sharding: core c -> batch b=c//2, query-row-half r=c%2 (1024 of 2048 rows).

v2 changes vs baseline (253us):
  - Host precomputes BOTH bias tables in the final SBUF gather layout:
      expF  = exp(bias)                      bf16  (ACT-path head pairs)
      schT  = round(A16*(bias-20) + B16)    int16  (Schraudolph head pairs)
    -> no device-side TW exp / DMA gather storm in the prologue.
  - Schraudolph softmax for SCH_PAIRS: scores arrive pre-scaled by
    A16=128/ln2 (folded into host Wq columns); ONE DVE tensor_add
    (psum f32 + int16 table -> int16) produces bf16 bits of
    exp(s+b-20) directly (bitcast), replacing ACT exp + DVE multiply.
    Softmax normalization cancels the common-mode approx error
    (validated end-to-end: rel ~6e-3 even with all heads approx).
  - ACT-path bias multiplies split DVE/GpSimd.
  - reciprocal_approx_fast (5x) instead of reciprocal in epilogues.
  - AV matmuls emitted in chunks of 3 kt to cut PE tiling-mode thrash.
  - Optional full-array warm-up matmul per chunk to keep HAM at 2.4GHz.
"""

import math
import sys

import numpy as np

if "/opt/trn_rl_repo" not in sys.path:
    sys.path.insert(0, "/opt/trn_rl_repo")

N = 2048
C = 256
NH = 8
HD = 32
B = 4
QR = 1024  # q rows per core
S_SHIFT = 20.0
A16 = 128.0 / math.log(2.0)
B16 = 127.0 * 128.0

SCH_PAIRS = (3,)          # head pairs (g2) on the Schraudolph path
ACT_PAIRS = tuple(g for g in range(4) if g not in SCH_PAIRS)
GPS_EVERY = 3             # every GPS_EVERY-th ACT-path multiply goes to GpSimd
WARM_MM = False           # full-array dummy matmul per 3-kt chunk (HAM warm)
                          # NOTE: True corrupts results on HW (drain of the dummy
                          # races the next matmul's start=True PSUM clear)

_NC = None


def _build_nc(dbg=False):
    import concourse.bass as bass
    import concourse.tile as tile
    from concourse import bacc, mybir
    from concourse.bass import ds, ts

    f32 = mybir.dt.float32
    bf16 = mybir.dt.bfloat16
    i16 = mybir.dt.int16
    EXP = mybir.ActivationFunctionType.Exp

    nA = len(ACT_PAIRS)
    nS = len(SCH_PAIRS)
    pair_slot = {}
    for j, g in enumerate(ACT_PAIRS):
        pair_slot[g] = j
    for j, g in enumerate(SCH_PAIRS):
        pair_slot[g] = j

    nc = bacc.Bacc(None, target_bir_lowering=False, debug=False)

    xT = nc.dram_tensor("xT", [C, N], bf16, kind="ExternalInput")
    xTq = nc.dram_tensor("xTq", [C, QR], bf16, kind="ExternalInput")
    Wq_d = nc.dram_tensor("Wq", [C, C], bf16, kind="ExternalInput")
    Wk_d = nc.dram_tensor("Wk", [C, C], bf16, kind="ExternalInput")
    Wv_d = nc.dram_tensor("Wv", [C, C], bf16, kind="ExternalInput")
    Wo_d = nc.dram_tensor("Wo", [C, C], bf16, kind="ExternalInput")
    expfT_d = None
    schT_d = None
    if nA:
        expfT_d = nc.dram_tensor("expfT", [128, nA * 8448], bf16, kind="ExternalInput")
    if nS:
        schT_d = nc.dram_tensor("schT", [128, nS * 8448], i16, kind="ExternalInput")
    out_d = nc.dram_tensor("out", [QR, C], f32, kind="ExternalOutput")
    dbg_t = {}
    if dbg:
        dbg_t["qT0"] = nc.dram_tensor("dbg_qT0", [128, QR], bf16, kind="ExternalOutput")
        dbg_t["kT0"] = nc.dram_tensor("dbg_kT0", [128, N], bf16, kind="ExternalOutput")
        dbg_t["qT1"] = nc.dram_tensor("dbg_qT1", [128, QR], bf16, kind="ExternalOutput")
        dbg_t["kT1"] = nc.dram_tensor("dbg_kT1", [128, N], bf16, kind="ExternalOutput")
        dbg_t["e_act"] = nc.dram_tensor("dbg_e_act", [128, 1024], bf16, kind="ExternalOutput")
        dbg_t["e_sch"] = nc.dram_tensor("dbg_e_sch", [128, 1024], i16, kind="ExternalOutput")
        dbg_t["s_act"] = nc.dram_tensor("dbg_s_act", [128, 1024], f32, kind="ExternalOutput")
        dbg_t["acc0"] = nc.dram_tensor("dbg_acc0", [128, 512], f32, kind="ExternalOutput")

    with tile.TileContext(nc) as tc:
        with (
            tc.tile_pool(name="consts", bufs=1) as consts,
            tc.tile_pool(name="tbl", bufs=1) as tbl,
            tc.tile_pool(name="xp", bufs=3) as xp,
            tc.tile_pool(name="kqv", bufs=1) as kqv,
            tc.tile_pool(name="ep", bufs=8) as ep,
            tc.tile_pool(name="rp", bufs=2) as rp,
            tc.tile_pool(name="otp", bufs=2) as otp,
            tc.tile_pool(name="stp", bufs=2) as stp,
            tc.tile_pool(name="spsum", bufs=3, space="PSUM") as spsum,
            tc.tile_pool(name="apsum", bufs=2, space="PSUM") as apsum,
        ):
            # ---- bias tables: straight DMA in final layout ----
            expf_view = None
            sch_view = None
            dma_engines = [nc.sync, nc.scalar, nc.gpsimd]
            dma_i = 0
            if nA:
                expf_sb = tbl.tile([128, nA * 8448], bf16, tag="expf")
                expf_view = expf_sb.rearrange(
                    "p (h r f) -> p h r f", h=2 * nA, r=11, f=384
                )
            if nS:
                sch_sb = tbl.tile([128, nS * 8448], i16, tag="sch")
                sch_view = sch_sb.rearrange(
                    "p (h r f) -> p h r f", h=2 * nS, r=11, f=384
                )
            # issue per-pair chunks in processing order g2=0..3
            for g2 in range(4):
                j = pair_slot[g2]
                if g2 in SCH_PAIRS:
                    src, dst = schT_d, sch_sb
                else:
                    src, dst = expfT_d, expf_sb
                eng = dma_engines[dma_i % len(dma_engines)]
                dma_i += 1
                eng.dma_start(
                    out=dst[:, ds(j * 8448, 8448)],
                    in_=src[:, ds(j * 8448, 8448)],
                )

            # ---- constants ----
            w_sb = {}
            for name, d in (("Wq", Wq_d), ("Wk", Wk_d), ("Wv", Wv_d), ("Wo", Wo_d)):
                t = consts.tile([128, 2, C], bf16, tag=f"w_{name}", name=f"w_{name}")
                nc.sync.dma_start(out=t, in_=d[:].rearrange("(ch p) n -> p ch n", p=128))
                w_sb[name] = t
            ones_sb = consts.tile([128, 32], bf16, tag="ones")
            nc.vector.memset(ones_sb, 1.0)
            ebias = consts.tile([128, 1], f32, tag="ebias")
            nc.vector.memset(ebias, -S_SHIFT)

            # ---- projections (all bf16; q scale folded into host Wq) ----
            kT_sb = [kqv.tile([128, N], bf16, tag=f"kT{m}", name=f"kT{m}")
                     for m in range(2)]
            qT_sb = [kqv.tile([128, QR], bf16, tag=f"qT{m}", name=f"qT{m}")
                     for m in range(2)]
            v_sb = kqv.tile([128, 16, C], bf16, tag="v")

            xTq_r = xTq[:].rearrange("(ch p) n -> p ch n", p=128)
            for j in range(QR // 512):
                xq = xp.tile([128, 2, 512], bf16, tag="x")
                nc.sync.dma_start(out=xq, in_=xTq_r[:, :, ds(512 * j, 512)])
                for m in range(2):
                    ps = spsum.tile([128, 1024], f32, tag="s")
                    for ch in range(2):
                        nc.tensor.matmul(
                            ps[:, :512],
                            lhsT=w_sb["Wq"][:, ch, ts(m, 128)],
                            rhs=xq[:, ch, :],
                            start=(ch == 0),
                            stop=(ch == 1),
                        )
                    nc.vector.tensor_copy(qT_sb[m][:, ds(512 * j, 512)], ps[:, :512])

            xT_r = xT[:].rearrange("(ch p) n -> p ch n", p=128)
            for j in range(N // 512):
                xc = xp.tile([128, 2, 512], bf16, tag="x")
                nc.sync.dma_start(out=xc, in_=xT_r[:, :, ds(512 * j, 512)])
                for m in range(2):
                    ps = spsum.tile([128, 1024], f32, tag="s")
                    for ch in range(2):
                        nc.tensor.matmul(
                            ps[:, :512],
                            lhsT=w_sb["Wk"][:, ch, ts(m, 128)],
                            rhs=xc[:, ch, :],
                            start=(ch == 0),
                            stop=(ch == 1),
                        )
                    nc.vector.tensor_copy(kT_sb[m][:, ds(512 * j, 512)], ps[:, :512])
                for t in range(4):
                    kt = 4 * j + t
                    ps = spsum.tile([128, 1024], f32, tag="s")
                    for ch in range(2):
                        nc.tensor.matmul(
                            ps[:, :C],
                            lhsT=xc[:, ch, ts(t, 128)],
                            rhs=w_sb["Wv"][:, ch, :],
                            start=(ch == 0),
                            stop=(ch == 1),
                        )
                    nc.vector.tensor_copy(v_sb[:, kt, :], ps[:, :C])

            if dbg:
                nc.sync.dma_start(out=dbg_t["qT0"][:], in_=qT_sb[0])
                nc.sync.dma_start(out=dbg_t["qT1"][:], in_=qT_sb[1])
                nc.sync.dma_start(out=dbg_t["kT0"][:], in_=kT_sb[0])
                nc.sync.dma_start(out=dbg_t["kT1"][:], in_=kT_sb[1])

            # ---- main attention loops ----
            oT_tiles = []
            for qc in range(2):
                oT = otp.tile([128, 2, 512], bf16, tag="oT", name=f"oT{qc}")
                oT_tiles.append(oT)
            mul_ctr = 0
            for g2 in range(4):
                is_sch = g2 in SCH_PAIRS
                slot = pair_slot[g2]
                for qc in range(2):
                    oT = oT_tiles[qc]
                    po_av = 0 if g2 % 2 == 0 else 64
                    po_rs = 64 - po_av
                    half_idx = g2 // 2
                    acc = apsum.tile([128, 512], f32, tag="acc")
                    e_tiles = {}

                    def emit_av(kt):
                        e_t, e_sch = e_tiles.pop(kt)
                        for k in range(2):
                            h = 2 * g2 + k
                            rhs = e_t[:, ts(k, 512)]
                            if e_sch:
                                rhs = rhs.bitcast(bf16)
                            nc.tensor.matmul(
                                acc[ds(po_av + 32 * k, 32), :],
                                lhsT=v_sb[:, kt, ds(32 * h, 32)],
                                rhs=rhs,
                                start=(kt == 0),
                                stop=(kt == 15),
                                tile_position=(0, po_av + 32 * k),
                                skip_group_check=True,
                            )
                            nc.tensor.matmul(
                                acc[ds(po_rs + 32 * k, 32), :],
                                lhsT=ones_sb,
                                rhs=rhs,
                                start=(kt == 0),
                                stop=(kt == 15),
                                tile_position=(0, po_rs + 32 * k),
                                skip_group_check=True,
                            )

                    for kt in range(16):
                        s_ps = spsum.tile([128, 1024], f32, tag="s")
                        if WARM_MM and kt % 3 == 0:
                            # full-array matmul to register PE activity with HAM
                            nc.tensor.matmul(
                                s_ps[:, :64],
                                lhsT=w_sb["Wo"][:, 0, ts(0, 128)],
                                rhs=w_sb["Wo"][:, 0, :64],
                                start=True,
                                stop=True,
                                skip_group_check=True,
                            )
                        for k in range(2):
                            h = 2 * g2 + k
                            i = h % 4
                            nc.tensor.matmul(
                                s_ps[:, ts(k, 512)],
                                lhsT=kT_sb[half_idx][ds(32 * i, 32), ts(kt, 128)],
                                rhs=qT_sb[half_idx][ds(32 * i, 32), ts(qc, 512)],
                                start=True,
                                stop=True,
                                tile_position=(32 * i, 0),
                            )
                        rdw0 = 2 * qc - (kt // 2) + 7
                        woff = 128 if kt % 2 == 0 else 0
                        if is_sch:
                            e_sb = ep.tile([128, 1024], i16, tag="e")
                            e4 = e_sb.rearrange("p (k jj f) -> p k jj f", k=2, jj=2)
                            s4 = s_ps.rearrange("p (k jj f) -> p k jj f", k=2, jj=2)
                            bt4 = sch_view[
                                :, ds(2 * slot, 2), ds(rdw0, 2), ds(woff, 256)
                            ]
                            nc.vector.tensor_add(e4, s4, bt4)
                            if dbg and g2 == SCH_PAIRS[0] and qc == 0 and kt == 0:
                                nc.sync.dma_start(out=dbg_t["e_sch"][:], in_=e_sb)
                            e_tiles[kt] = (e_sb, True)
                        else:
                            e_sb = ep.tile([128, 1024], bf16, tag="e")
                            nc.scalar.activation(e_sb, s_ps, EXP, bias=ebias[:, :])
                            ev = e_sb.rearrange("p (k jj f) -> p k jj f", k=2, jj=2)
                            fv = expf_view[
                                :, ds(2 * slot, 2), ds(rdw0, 2), ds(woff, 256)
                            ]
                            eng = (
                                nc.gpsimd
                                if (GPS_EVERY and mul_ctr % GPS_EVERY == GPS_EVERY - 1)
                                else nc.vector
                            )
                            mul_ctr += 1
                            if dbg and g2 == 0 and qc == 0 and kt == 0:
                                dstage = stp.tile([128, 1024], f32, tag="dst")
                                nc.vector.tensor_copy(dstage, s_ps)
                                nc.sync.dma_start(out=dbg_t["s_act"][:], in_=dstage)
                            eng.tensor_mul(ev, ev, fv)
                            if dbg and g2 == 0 and qc == 0 and kt == 0:
                                nc.scalar.dma_start(out=dbg_t["e_act"][:], in_=e_sb)
                            e_tiles[kt] = (e_sb, False)
                        if kt in (5, 8, 11, 14):
                            for k2 in range(kt - 5, kt - 2):
                                emit_av(k2)
                    for k2 in (12, 13, 14, 15):
                        emit_av(k2)
                    if dbg and g2 == 0 and qc == 0:
                        astage = stp.tile([128, 512], f32, tag="ast")
                        nc.vector.tensor_copy(astage, acc)
                        nc.sync.dma_start(out=dbg_t["acc0"][:], in_=astage)
                    # epilogue: normalize 2 heads into oT.
                    # reciprocal_approx_fast must run at partition base 0 on
                    # HW (base-64 invocations corrupt results; sim is fine).
                    recip = rp.tile([128, 512], f32, tag="recip")
                    rep = rp.tile([128, 512], f32, tag="rep")
                    if po_rs == 0:
                        nc.vector.reciprocal_approx_fast(
                            recip[ds(0, 64), :], acc[ds(0, 64), :]
                        )
                    else:
                        nc.vector.tensor_copy(
                            rep[ds(64, 64), :], acc[ds(64, 64), :]
                        )
                        nc.sync.dma_start(
                            out=rep[ds(0, 64), :], in_=rep[ds(64, 64), :]
                        )
                        nc.vector.reciprocal_approx_fast(
                            recip[ds(0, 64), :], rep[ds(0, 64), :]
                        )
                    if po_av == 0:
                        nc.vector.tensor_mul(
                            oT[ds(0, 64), half_idx, :],
                            acc[ds(0, 64), :],
                            recip[ds(0, 64), :],
                        )
                    else:
                        nc.sync.dma_start(
                            out=rep[ds(64, 64), :], in_=recip[ds(0, 64), :]
                        )
                        nc.vector.tensor_mul(
                            oT[ds(64, 64), half_idx, :],
                            acc[ds(64, 64), :],
                            rep[ds(64, 64), :],
                        )
            # final projections
            for qc in range(2):
                oT = oT_tiles[qc]
                for s in range(4):
                    fps = spsum.tile([128, 1024], f32, tag="s")
                    for ch in range(2):
                        nc.tensor.matmul(
                            fps[:, :C],
                            lhsT=oT[:, ch, ts(s, 128)],
                            rhs=w_sb["Wo"][:, ch, :],
                            start=(ch == 0),
                            stop=(ch == 1),
                        )
                    stage = stp.tile([128, C], f32, tag="stage")
                    nc.vector.tensor_copy(stage, fps[:, :C])
                    nc.sync.dma_start(
                        out=out_d[ds(512 * qc + 128 * s, 128), :], in_=stage
                    )

    nc.compile()
    return nc


def _host_tables(T):
    """Per-row-half bias tables in the final SBUF gather layout.

    Returns {r: (expf bf16 [128, nA*8448], sch int16 [128, nS*8448])}.
    Layout: partition p = 16*h2p + w2, free = (pair-slot-local head, rdw 11,
    f 384) where f = 16*drh + w1, gathered value
    G[p,h,rdw,f] = bias_table[h, 4r+rdw, (7-h2p)+drh, 15+w1-w2].
    """
    import ml_dtypes

    bf = ml_dtypes.bfloat16
    T = np.asarray(T, dtype=np.float32)
    p = np.arange(128)
    h2p, w2 = p // 16, p % 16
    f = np.arange(384)
    drh, w1 = f // 16, f % 16
    rh = (7 - h2p)[:, None] + drh[None, :]          # [128, 384]
    rw = 15 + w1[None, :] - w2[:, None]             # [128, 384]
    out = {}
    for r in (0, 1):
        Twin = T[:, 4 * r:4 * r + 11]               # [8, 11, 31, 31]
        G = Twin[:, :, rh, rw]                      # [8, 11, 128, 384]
        G = np.ascontiguousarray(G.transpose(2, 0, 1, 3))  # [128, 8, 11, 384]
        expf = None
        sch = None
        if ACT_PAIRS:
            heads = []
            for g in ACT_PAIRS:
                heads += [2 * g, 2 * g + 1]
            expf = np.ascontiguousarray(
                np.exp(G[:, heads]).reshape(128, -1).astype(bf)
            )
        if SCH_PAIRS:
            heads = []
            for g in SCH_PAIRS:
                heads += [2 * g, 2 * g + 1]
            sch = np.ascontiguousarray(
                np.round(A16 * (G[:, heads] - S_SHIFT) + B16)
                .reshape(128, -1).astype(np.int16)
            )
        out[r] = (expf, sch)
    return out


def _host_inputs(x, Wq, Wk, Wv, Wo, bias_table):
    """Build the 8 per-core input maps."""
    import ml_dtypes

    bf = ml_dtypes.bfloat16
    x = np.asarray(x, dtype=np.float32)
    xf = np.ascontiguousarray(x.reshape(B, N, C))
    qsc = 1.0 / math.sqrt(HD)
    scale = np.full(NH, qsc, np.float32)
    for g in SCH_PAIRS:
        scale[2 * g] = qsc * A16
        scale[2 * g + 1] = qsc * A16
    Wq_s = np.asarray(Wq, np.float32).reshape(C, NH, HD) * scale[None, :, None]
    Ws = {
        "Wq": np.ascontiguousarray(Wq_s.reshape(C, C).astype(bf)),
        "Wk": np.ascontiguousarray(np.asarray(Wk, np.float32).astype(bf)),
        "Wv": np.ascontiguousarray(np.asarray(Wv, np.float32).astype(bf)),
        "Wo": np.ascontiguousarray(np.asarray(Wo, np.float32).astype(bf)),
    }
    tables = _host_tables(bias_table)
    in_maps = []
    for c in range(8):
        b, r = c // 2, c % 2
        expf, sch = tables[r]
        m = {
            "xT": np.ascontiguousarray(xf[b].T.astype(bf)),
            "xTq": np.ascontiguousarray(xf[b, QR * r:QR * (r + 1)].T.astype(bf)),
            **Ws,
        }
        if expf is not None:
            m["expfT"] = expf
        if sch is not None:
            m["schT"] = sch
        in_maps.append(m)
    return in_maps


def kernel(x, Wq, Wk, Wv, Wo, bias_table, _results_hook=None):
    global _NC
    if _NC is None:
        _NC = _build_nc()
    from concourse.bass_utils import run_bass_kernel_spmd

    in_maps = _host_inputs(x, Wq, Wk, Wv, Wo, bias_table)
    res = run_bass_kernel_spmd(_NC, in_maps, core_ids=list(range(8)))
    if _results_hook is not None:
        _results_hook(res)
    out = np.zeros((B, N, C), dtype=np.float32)
    for c in range(8):
        b, r = c // 2, c % 2
        out[b, QR * r:QR * (r + 1)] = res.results[c]["out"]
    D, H, W = 8, 16, 16
    return out.reshape(B, D, H, W, C)


# revision 19
# speedup vs baseline: 1.2033x; 1.0764x over previous
"""BrokenBiasAttention Trainium2 kernel (8-core SPMD), v2.

Sharding: core c -> batch b=c//2, query-row-half r=c%2 (1024 of 2048 rows).

v2 changes vs baseline (253us):
  - Host precomputes BOTH bias tables in the final SBUF gather layout:
      expF  = exp(bias)                      bf16  (ACT-path head pairs)
      schT  = round(A16*(bias-20) + B16)    int16  (Schraudolph head pairs)
    -> no device-side TW exp / DMA gather storm in the prologue.
  - Schraudolph softmax for SCH_PAIRS: scores arrive pre-scaled by
    A16=128/ln2 (folded into host Wq columns); ONE DVE tensor_add
    (psum f32 + int16 table -> int16) produces bf16 bits of
    exp(s+b-20) directly (bitcast), replacing ACT exp + DVE multiply.
    Softmax normalization cancels the common-mode approx error
    (validated end-to-end: rel ~6e-3 even with all heads approx).
  - ACT-path bias multiplies split DVE/GpSimd.
  - reciprocal_approx_fast (5x) instead of reciprocal in epilogues.
  - AV matmuls emitted in chunks of 3 kt to cut PE tiling-mode thrash.
  - Optional full-array warm-up matmul per chunk to keep HAM at 2.4GHz.
"""

import math
import sys

import numpy as np

if "/opt/trn_rl_repo" not in sys.path:
    sys.path.insert(0, "/opt/trn_rl_repo")

N = 2048
C = 256
NH = 8
HD = 32
B = 4
QR = 1024  # q rows per core
S_SHIFT = 20.0
A16 = 128.0 / math.log(2.0)
B16 = 127.0 * 128.0

SCH_PAIRS = (3,)          # head pairs (g2) on the Schraudolph path
ACT_PAIRS = tuple(g for g in range(4) if g not in SCH_PAIRS)
GPS_EVERY = 0             # every GPS_EVERY-th ACT-path multiply goes to GpSimd
                          # (0 = off: gpsimd muls are 2.2us AND contend for the
                          # shared SBUF port, inflating concurrent DVE ops)
WARM_MM = False           # full-array dummy matmul per 3-kt chunk (HAM warm)
                          # NOTE: True corrupts results on HW (drain of the dummy
                          # races the next matmul's start=True PSUM clear)

_NC = None


def _build_nc(dbg=False):
    import concourse.bass as bass
    import concourse.tile as tile
    from concourse import bacc, mybir
    from concourse.bass import ds, ts

    f32 = mybir.dt.float32
    bf16 = mybir.dt.bfloat16
    i16 = mybir.dt.int16
    EXP = mybir.ActivationFunctionType.Exp

    nA = len(ACT_PAIRS)
    nS = len(SCH_PAIRS)
    pair_slot = {}
    for j, g in enumerate(ACT_PAIRS):
        pair_slot[g] = j
    for j, g in enumerate(SCH_PAIRS):
        pair_slot[g] = j

    nc = bacc.Bacc(None, target_bir_lowering=False, debug=False)

    xT = nc.dram_tensor("xT", [C, N], bf16, kind="ExternalInput")
    xTq = nc.dram_tensor("xTq", [C, QR], bf16, kind="ExternalInput")
    Wq_d = nc.dram_tensor("Wq", [C, C], bf16, kind="ExternalInput")
    Wk_d = nc.dram_tensor("Wk", [C, C], bf16, kind="ExternalInput")
    Wv_d = nc.dram_tensor("Wv", [C, C], bf16, kind="ExternalInput")
    Wo_d = nc.dram_tensor("Wo", [C, C], bf16, kind="ExternalInput")
    expfT_d = None
    schT_d = None
    if nA:
        expfT_d = nc.dram_tensor("expfT", [128, nA * 8448], bf16, kind="ExternalInput")
    if nS:
        schT_d = nc.dram_tensor("schT", [128, nS * 8448], i16, kind="ExternalInput")
    out_d = nc.dram_tensor("out", [QR, C], f32, kind="ExternalOutput")
    dbg_t = {}
    if dbg:
        dbg_t["qT0"] = nc.dram_tensor("dbg_qT0", [128, QR], bf16, kind="ExternalOutput")
        dbg_t["kT0"] = nc.dram_tensor("dbg_kT0", [128, N], bf16, kind="ExternalOutput")
        dbg_t["qT1"] = nc.dram_tensor("dbg_qT1", [128, QR], bf16, kind="ExternalOutput")
        dbg_t["kT1"] = nc.dram_tensor("dbg_kT1", [128, N], bf16, kind="ExternalOutput")
        dbg_t["e_act"] = nc.dram_tensor("dbg_e_act", [128, 1024], bf16, kind="ExternalOutput")
        dbg_t["e_sch"] = nc.dram_tensor("dbg_e_sch", [128, 1024], i16, kind="ExternalOutput")
        dbg_t["s_act"] = nc.dram_tensor("dbg_s_act", [128, 1024], f32, kind="ExternalOutput")
        dbg_t["acc0"] = nc.dram_tensor("dbg_acc0", [128, 512], f32, kind="ExternalOutput")

    with tile.TileContext(nc) as tc:
        with (
            tc.tile_pool(name="consts", bufs=1) as consts,
            tc.tile_pool(name="tbl", bufs=1) as tbl,
            tc.tile_pool(name="xp", bufs=3) as xp,
            tc.tile_pool(name="kqv", bufs=1) as kqv,
            tc.tile_pool(name="ep", bufs=8) as ep,
            tc.tile_pool(name="rp", bufs=2) as rp,
            tc.tile_pool(name="otp", bufs=2) as otp,
            tc.tile_pool(name="stp", bufs=2) as stp,
            tc.tile_pool(name="spsum", bufs=3, space="PSUM") as spsum,
            tc.tile_pool(name="apsum", bufs=2, space="PSUM") as apsum,
        ):
            # ---- bias tables: straight DMA in final layout ----
            # keep these OFF the sync queue: weights/x DMAs (sync) must not
            # queue behind 8.6MB of table traffic
            expf_view = None
            sch_view = None
            dma_engines = [nc.scalar, nc.gpsimd]
            dma_i = 0
            if nA:
                expf_sb = tbl.tile([128, nA * 8448], bf16, tag="expf")
                expf_view = expf_sb.rearrange(
                    "p (h r f) -> p h r f", h=2 * nA, r=11, f=384
                )
            if nS:
                sch_sb = tbl.tile([128, nS * 8448], i16, tag="sch")
                sch_view = sch_sb.rearrange(
                    "p (h r f) -> p h r f", h=2 * nS, r=11, f=384
                )
            # issue per-pair chunks in block processing order
            for g2 in (0, 3, 1, 2):
                j = pair_slot[g2]
                if g2 in SCH_PAIRS:
                    src, dst = schT_d, sch_sb
                else:
                    src, dst = expfT_d, expf_sb
                eng = dma_engines[dma_i % len(dma_engines)]
                dma_i += 1
                eng.dma_start(
                    out=dst[:, ds(j * 8448, 8448)],
                    in_=src[:, ds(j * 8448, 8448)],
                )

            # ---- constants ----
            w_sb = {}
            for name, d in (("Wq", Wq_d), ("Wk", Wk_d), ("Wv", Wv_d), ("Wo", Wo_d)):
                t = consts.tile([128, 2, C], bf16, tag=f"w_{name}", name=f"w_{name}")
                nc.sync.dma_start(out=t, in_=d[:].rearrange("(ch p) n -> p ch n", p=128))
                w_sb[name] = t
            ones_sb = consts.tile([128, 32], bf16, tag="ones")
            nc.vector.memset(ones_sb, 1.0)
            ebias = consts.tile([128, 1], f32, tag="ebias")
            nc.vector.memset(ebias, -S_SHIFT)

            # ---- projections (all bf16; q scale folded into host Wq) ----
            kT_sb = [kqv.tile([128, N], bf16, tag=f"kT{m}", name=f"kT{m}")
                     for m in range(2)]
            qT_sb = [kqv.tile([128, QR], bf16, tag=f"qT{m}", name=f"qT{m}")
                     for m in range(2)]
            v_sb = kqv.tile([128, 16, C], bf16, tag="v")

            xTq_r = xTq[:].rearrange("(ch p) n -> p ch n", p=128)
            for j in range(QR // 512):
                xq = xp.tile([128, 2, 512], bf16, tag="x")
                nc.sync.dma_start(out=xq, in_=xTq_r[:, :, ds(512 * j, 512)])
                for m in range(2):
                    ps = spsum.tile([128, 1024], f32, tag="s")
                    for ch in range(2):
                        nc.tensor.matmul(
                            ps[:, :512],
                            lhsT=w_sb["Wq"][:, ch, ts(m, 128)],
                            rhs=xq[:, ch, :],
                            start=(ch == 0),
                            stop=(ch == 1),
                        )
                    nc.scalar.copy(qT_sb[m][:, ds(512 * j, 512)], ps[:, :512])

            xT_r = xT[:].rearrange("(ch p) n -> p ch n", p=128)
            for j in range(N // 512):
                xc = xp.tile([128, 2, 512], bf16, tag="x")
                nc.sync.dma_start(out=xc, in_=xT_r[:, :, ds(512 * j, 512)])
                for m in range(2):
                    ps = spsum.tile([128, 1024], f32, tag="s")
                    for ch in range(2):
                        nc.tensor.matmul(
                            ps[:, :512],
                            lhsT=w_sb["Wk"][:, ch, ts(m, 128)],
                            rhs=xc[:, ch, :],
                            start=(ch == 0),
                            stop=(ch == 1),
                        )
                    nc.scalar.copy(kT_sb[m][:, ds(512 * j, 512)], ps[:, :512])
                for t in range(4):
                    kt = 4 * j + t
                    ps = spsum.tile([128, 1024], f32, tag="s")
                    for ch in range(2):
                        nc.tensor.matmul(
                            ps[:, :C],
                            lhsT=xc[:, ch, ts(t, 128)],
                            rhs=w_sb["Wv"][:, ch, :],
                            start=(ch == 0),
                            stop=(ch == 1),
                        )
                    nc.scalar.copy(v_sb[:, kt, :], ps[:, :C])

            if dbg:
                nc.sync.dma_start(out=dbg_t["qT0"][:], in_=qT_sb[0])
                nc.sync.dma_start(out=dbg_t["qT1"][:], in_=qT_sb[1])
                nc.sync.dma_start(out=dbg_t["kT0"][:], in_=kT_sb[0])
                nc.sync.dma_start(out=dbg_t["kT1"][:], in_=kT_sb[1])

            # ---- main attention loops ----
            oT_tiles = []
            for qc in range(2):
                oT = otp.tile([128, 2, 512], bf16, tag="oT", name=f"oT{qc}")
                oT_tiles.append(oT)
            mul_ctr = 0
            # SCH blocks interleaved mid-sequence: their DVE-serial work
            # overlaps the neighboring ACT blocks' exp stream
            block_order = [(0, 0), (0, 1), (3, 0), (1, 0), (1, 1), (3, 1),
                           (2, 0), (2, 1)]
            if SCH_PAIRS != (3,):
                block_order = [(g, qc) for g in range(4) for qc in range(2)]
            for g2, qc in block_order:
                is_sch = g2 in SCH_PAIRS
                slot = pair_slot[g2]
                if True:
                    oT = oT_tiles[qc]
                    po_av = 0 if g2 % 2 == 0 else 64
                    po_rs = 64 - po_av
                    half_idx = g2 // 2
                    acc = apsum.tile([128, 512], f32, tag="acc")
                    e_tiles = {}

                    def emit_av(kt):
                        e_t, e_sch = e_tiles.pop(kt)
                        for k in range(2):
                            h = 2 * g2 + k
                            rhs = e_t[:, ts(k, 512)]
                            if e_sch:
                                rhs = rhs.bitcast(bf16)
                            nc.tensor.matmul(
                                acc[ds(po_av + 32 * k, 32), :],
                                lhsT=v_sb[:, kt, ds(32 * h, 32)],
                                rhs=rhs,
                                start=(kt == 0),
                                stop=(kt == 15),
                                tile_position=(0, po_av + 32 * k),
                                skip_group_check=True,
                            )
                            nc.tensor.matmul(
                                acc[ds(po_rs + 32 * k, 32), :],
                                lhsT=ones_sb,
                                rhs=rhs,
                                start=(kt == 0),
                                stop=(kt == 15),
                                tile_position=(0, po_rs + 32 * k),
                                skip_group_check=True,
                            )

                    for kt in range(16):
                        s_ps = spsum.tile([128, 1024], f32, tag="s")
                        if WARM_MM and kt % 3 == 0:
                            # full-array matmul to register PE activity with HAM
                            nc.tensor.matmul(
                                s_ps[:, :64],
                                lhsT=w_sb["Wo"][:, 0, ts(0, 128)],
                                rhs=w_sb["Wo"][:, 0, :64],
                                start=True,
                                stop=True,
                                skip_group_check=True,
                            )
                        for k in range(2):
                            h = 2 * g2 + k
                            i = h % 4
                            nc.tensor.matmul(
                                s_ps[:, ts(k, 512)],
                                lhsT=kT_sb[half_idx][ds(32 * i, 32), ts(kt, 128)],
                                rhs=qT_sb[half_idx][ds(32 * i, 32), ts(qc, 512)],
                                start=True,
                                stop=True,
                                tile_position=(32 * i, 0),
                            )
                        rdw0 = 2 * qc - (kt // 2) + 7
                        woff = 128 if kt % 2 == 0 else 0
                        if is_sch:
                            e_sb = ep.tile([128, 1024], i16, tag="e")
                            e4 = e_sb.rearrange("p (k jj f) -> p k jj f", k=2, jj=2)
                            s4 = s_ps.rearrange("p (k jj f) -> p k jj f", k=2, jj=2)
                            bt4 = sch_view[
                                :, ds(2 * slot, 2), ds(rdw0, 2), ds(woff, 256)
                            ]
                            nc.vector.tensor_add(e4, s4, bt4)
                            if dbg and g2 == SCH_PAIRS[0] and qc == 0 and kt == 0:
                                nc.sync.dma_start(out=dbg_t["e_sch"][:], in_=e_sb)
                            e_tiles[kt] = (e_sb, True)
                        else:
                            e_sb = ep.tile([128, 1024], bf16, tag="e")
                            nc.scalar.activation(e_sb, s_ps, EXP, bias=ebias[:, :])
                            ev = e_sb.rearrange("p (k jj f) -> p k jj f", k=2, jj=2)
                            fv = expf_view[
                                :, ds(2 * slot, 2), ds(rdw0, 2), ds(woff, 256)
                            ]
                            eng = (
                                nc.gpsimd
                                if (GPS_EVERY and mul_ctr % GPS_EVERY == GPS_EVERY - 1)
                                else nc.vector
                            )
                            mul_ctr += 1
                            if dbg and g2 == 0 and qc == 0 and kt == 0:
                                dstage = stp.tile([128, 1024], f32, tag="dst")
                                nc.vector.tensor_copy(dstage, s_ps)
                                nc.sync.dma_start(out=dbg_t["s_act"][:], in_=dstage)
                            eng.tensor_mul(ev, ev, fv)
                            if dbg and g2 == 0 and qc == 0 and kt == 0:
                                nc.scalar.dma_start(out=dbg_t["e_act"][:], in_=e_sb)
                            e_tiles[kt] = (e_sb, False)
                        if kt in (5, 8, 11, 14):
                            for k2 in range(kt - 5, kt - 2):
                                emit_av(k2)
                    for k2 in (12, 13, 14, 15):
                        emit_av(k2)
                    if dbg and g2 == 0 and qc == 0:
                        astage = stp.tile([128, 512], f32, tag="ast")
                        nc.vector.tensor_copy(astage, acc)
                        nc.sync.dma_start(out=dbg_t["acc0"][:], in_=astage)
                    # epilogue: normalize 2 heads into oT.
                    # reciprocal_approx_fast must run at partition base 0 on
                    # HW (base-64 invocations corrupt results; sim is fine).
                    recip = rp.tile([128, 512], f32, tag="recip")
                    rep = rp.tile([128, 512], f32, tag="rep")
                    if po_rs == 0:
                        nc.vector.reciprocal_approx_fast(
                            recip[ds(0, 64), :], acc[ds(0, 64), :]
                        )
                    else:
                        nc.vector.tensor_copy(
                            rep[ds(64, 64), :], acc[ds(64, 64), :]
                        )
                        nc.sync.dma_start(
                            out=rep[ds(0, 64), :], in_=rep[ds(64, 64), :]
                        )
                        nc.vector.reciprocal_approx_fast(
                            recip[ds(0, 64), :], rep[ds(0, 64), :]
                        )
                    if po_av == 0:
                        nc.vector.tensor_mul(
                            oT[ds(0, 64), half_idx, :],
                            acc[ds(0, 64), :],
                            recip[ds(0, 64), :],
                        )
                    else:
                        nc.sync.dma_start(
                            out=rep[ds(64, 64), :], in_=recip[ds(0, 64), :]
                        )
                        nc.vector.tensor_mul(
                            oT[ds(64, 64), half_idx, :],
                            acc[ds(64, 64), :],
                            rep[ds(64, 64), :],
                        )
            # final projections
            for qc in range(2):
                oT = oT_tiles[qc]
                for s in range(4):
                    fps = spsum.tile([128, 1024], f32, tag="s")
                    for ch in range(2):
                        nc.tensor.matmul(
                            fps[:, :C],
                            lhsT=oT[:, ch, ts(s, 128)],
                            rhs=w_sb["Wo"][:, ch, :],
                            start=(ch == 0),
                            stop=(ch == 1),
                        )
                    stage = stp.tile([128, C], f32, tag="stage")
                    nc.scalar.copy(stage, fps[:, :C])
                    nc.sync.dma_start(
                        out=out_d[ds(512 * qc + 128 * s, 128), :], in_=stage
                    )

    nc.compile()
    return nc


def _host_tables(T):
    """Per-row-half bias tables in the final SBUF gather layout.

    Returns {r: (expf bf16 [128, nA*8448], sch int16 [128, nS*8448])}.
    Layout: partition p = 16*h2p + w2, free = (pair-slot-local head, rdw 11,
    f 384) where f = 16*drh + w1, gathered value
    G[p,h,rdw,f] = bias_table[h, 4r+rdw, (7-h2p)+drh, 15+w1-w2].
    """
    import ml_dtypes

    bf = ml_dtypes.bfloat16
    T = np.asarray(T, dtype=np.float32)
    p = np.arange(128)
    h2p, w2 = p // 16, p % 16
    f = np.arange(384)
    drh, w1 = f // 16, f % 16
    rh = (7 - h2p)[:, None] + drh[None, :]          # [128, 384]
    rw = 15 + w1[None, :] - w2[:, None]             # [128, 384]
    out = {}
    for r in (0, 1):
        Twin = T[:, 4 * r:4 * r + 11]               # [8, 11, 31, 31]
        G = Twin[:, :, rh, rw]                      # [8, 11, 128, 384]
        G = np.ascontiguousarray(G.transpose(2, 0, 1, 3))  # [128, 8, 11, 384]
        expf = None
        sch = None
        if ACT_PAIRS:
            heads = []
            for g in ACT_PAIRS:
                heads += [2 * g, 2 * g + 1]
            expf = np.ascontiguousarray(
                np.exp(G[:, heads]).reshape(128, -1).astype(bf)
            )
        if SCH_PAIRS:
            heads = []
            for g in SCH_PAIRS:
                heads += [2 * g, 2 * g + 1]
            sch = np.ascontiguousarray(
                np.round(A16 * (G[:, heads] - S_SHIFT) + B16)
                .reshape(128, -1).astype(np.int16)
            )
        out[r] = (expf, sch)
    return out


def _host_inputs(x, Wq, Wk, Wv, Wo, bias_table):
    """Build the 8 per-core input maps."""
    import ml_dtypes

    bf = ml_dtypes.bfloat16
    x = np.asarray(x, dtype=np.float32)
    xf = np.ascontiguousarray(x.reshape(B, N, C))
    qsc = 1.0 / math.sqrt(HD)
    scale = np.full(NH, qsc, np.float32)
    for g in SCH_PAIRS:
        scale[2 * g] = qsc * A16
        scale[2 * g + 1] = qsc * A16
    Wq_s = np.asarray(Wq, np.float32).reshape(C, NH, HD) * scale[None, :, None]
    Ws = {
        "Wq": np.ascontiguousarray(Wq_s.reshape(C, C).astype(bf)),
        "Wk": np.ascontiguousarray(np.asarray(Wk, np.float32).astype(bf)),
        "Wv": np.ascontiguousarray(np.asarray(Wv, np.float32).astype(bf)),
        "Wo": np.ascontiguousarray(np.asarray(Wo, np.float32).astype(bf)),
    }
    tables = _host_tables(bias_table)
    in_maps = []
    for c in range(8):
        b, r = c // 2, c % 2
        expf, sch = tables[r]
        m = {
            "xT": np.ascontiguousarray(xf[b].T.astype(bf)),
            "xTq": np.ascontiguousarray(xf[b, QR * r:QR * (r + 1)].T.astype(bf)),
            **Ws,
        }
        if expf is not None:
            m["expfT"] = expf
        if sch is not None:
            m["schT"] = sch
        in_maps.append(m)
    return in_maps


def kernel(x, Wq, Wk, Wv, Wo, bias_table, _results_hook=None):
    global _NC
    if _NC is None:
        _NC = _build_nc()
    from concourse.bass_utils import run_bass_kernel_spmd

    in_maps = _host_inputs(x, Wq, Wk, Wv, Wo, bias_table)
    res = run_bass_kernel_spmd(_NC, in_maps, core_ids=list(range(8)))
    if _results_hook is not None:
        _results_hook(res)
    out = np.zeros((B, N, C), dtype=np.float32)
    for c in range(8):
        b, r = c // 2, c % 2
        out[b, QR * r:QR * (r + 1)] = res.results[c]["out"]
    D, H, W = 8, 16, 16
    return out.reshape(B, D, H, W, C)


# revision 29
# speedup vs baseline: 1.2243x; 1.0174x over previous
"""BrokenBiasAttention Trainium2 kernel (8-core SPMD), v2.

Sharding: core c -> batch b=c//2, query-row-half r=c%2 (1024 of 2048 rows).

v2 changes vs baseline (253us):
  - Host precomputes BOTH bias tables in the final SBUF gather layout:
      expF  = exp(bias)                      bf16  (ACT-path head pairs)
      schT  = round(A16*(bias-20) + B16)    int16  (Schraudolph head pairs)
    -> no device-side TW exp / DMA gather storm in the prologue.
  - Schraudolph softmax for SCH_PAIRS: scores arrive pre-scaled by
    A16=128/ln2 (folded into host Wq columns); ONE DVE tensor_add
    (psum f32 + int16 table -> int16) produces bf16 bits of
    exp(s+b-20) directly (bitcast), replacing ACT exp + DVE multiply.
    Softmax normalization cancels the common-mode approx error
    (validated end-to-end: rel ~6e-3 even with all heads approx).
  - ACT-path bias multiplies split DVE/GpSimd.
  - reciprocal_approx_fast (5x) instead of reciprocal in epilogues.
  - AV matmuls emitted in chunks of 3 kt to cut PE tiling-mode thrash.
  - Optional full-array warm-up matmul per chunk to keep HAM at 2.4GHz.
"""

import math
import sys

import numpy as np

if "/opt/trn_rl_repo" not in sys.path:
    sys.path.insert(0, "/opt/trn_rl_repo")

N = 2048
C = 256
NH = 8
HD = 32
B = 4
QR = 1024  # q rows per core
S_SHIFT = 20.0
A16 = 128.0 / math.log(2.0)
B16 = 127.0 * 128.0

SCH_PAIRS = (3,)          # head pairs (g2) on the Schraudolph path
ACT_PAIRS = tuple(g for g in range(4) if g not in SCH_PAIRS)
GPS_EVERY = 0             # every GPS_EVERY-th ACT-path multiply goes to GpSimd
                          # (0 = off: gpsimd muls are 2.2us AND contend for the
                          # shared SBUF port, inflating concurrent DVE ops)
WARM_MM = False           # full-array dummy matmul per 3-kt chunk (HAM warm)
                          # NOTE: True corrupts results on HW (drain of the dummy
                          # races the next matmul's start=True PSUM clear)

_NC = None


def _build_nc(dbg=False):
    import concourse.bass as bass
    import concourse.tile as tile
    from concourse import bacc, mybir
    from concourse.bass import ds, ts

    f32 = mybir.dt.float32
    bf16 = mybir.dt.bfloat16
    i16 = mybir.dt.int16
    EXP = mybir.ActivationFunctionType.Exp

    nA = len(ACT_PAIRS)
    nS = len(SCH_PAIRS)
    pair_slot = {}
    for j, g in enumerate(ACT_PAIRS):
        pair_slot[g] = j
    for j, g in enumerate(SCH_PAIRS):
        pair_slot[g] = j

    nc = bacc.Bacc(None, target_bir_lowering=False, debug=False)

    xT = nc.dram_tensor("xT", [C, N], bf16, kind="ExternalInput")
    xTq = nc.dram_tensor("xTq", [C, QR], bf16, kind="ExternalInput")
    Wq_d = nc.dram_tensor("Wq", [C, C], bf16, kind="ExternalInput")
    Wk_d = nc.dram_tensor("Wk", [C, C], bf16, kind="ExternalInput")
    Wv_d = nc.dram_tensor("Wv", [C, C], bf16, kind="ExternalInput")
    Wo_d = nc.dram_tensor("Wo", [C, C], bf16, kind="ExternalInput")
    expfT_d = None
    schT_d = None
    if nA:
        expfT_d = nc.dram_tensor("expfT", [128, nA * 8448], bf16, kind="ExternalInput")
    if nS:
        schT_d = nc.dram_tensor("schT", [128, nS * 8448], i16, kind="ExternalInput")
    out_d = nc.dram_tensor("out", [QR, C], f32, kind="ExternalOutput")
    dbg_t = {}
    if dbg:
        dbg_t["qT0"] = nc.dram_tensor("dbg_qT0", [128, QR], bf16, kind="ExternalOutput")
        dbg_t["kT0"] = nc.dram_tensor("dbg_kT0", [128, N], bf16, kind="ExternalOutput")
        dbg_t["qT1"] = nc.dram_tensor("dbg_qT1", [128, QR], bf16, kind="ExternalOutput")
        dbg_t["kT1"] = nc.dram_tensor("dbg_kT1", [128, N], bf16, kind="ExternalOutput")
        dbg_t["e_act"] = nc.dram_tensor("dbg_e_act", [128, 1024], bf16, kind="ExternalOutput")
        dbg_t["e_sch"] = nc.dram_tensor("dbg_e_sch", [128, 1024], i16, kind="ExternalOutput")
        dbg_t["s_act"] = nc.dram_tensor("dbg_s_act", [128, 1024], f32, kind="ExternalOutput")
        dbg_t["acc0"] = nc.dram_tensor("dbg_acc0", [128, 512], f32, kind="ExternalOutput")

    with tile.TileContext(nc) as tc:
        with (
            tc.tile_pool(name="consts", bufs=1) as consts,
            tc.tile_pool(name="tbl", bufs=1) as tbl,
            tc.tile_pool(name="xp", bufs=3) as xp,
            tc.tile_pool(name="kqv", bufs=1) as kqv,
            tc.tile_pool(name="ep", bufs=8) as ep,
            tc.tile_pool(name="rp", bufs=2) as rp,
            tc.tile_pool(name="otp", bufs=2) as otp,
            tc.tile_pool(name="stp", bufs=2) as stp,
            tc.tile_pool(name="spsum", bufs=3, space="PSUM") as spsum,
            tc.tile_pool(name="apsum", bufs=2, space="PSUM") as apsum,
        ):
            # ---- bias tables: straight DMA in final layout ----
            # keep these OFF the sync queue: weights/x DMAs (sync) must not
            # queue behind 8.6MB of table traffic
            expf_view = None
            sch_view = None
            dma_engines = [nc.scalar, nc.gpsimd]
            dma_i = 0
            if nA:
                expf_sb = tbl.tile([128, nA * 8448], bf16, tag="expf")
                expf_view = expf_sb.rearrange(
                    "p (h r f) -> p h r f", h=2 * nA, r=11, f=384
                )
            if nS:
                sch_sb = tbl.tile([128, nS * 8448], i16, tag="sch")
                sch_view = sch_sb.rearrange(
                    "p (h r f) -> p h r f", h=2 * nS, r=11, f=384
                )
            def emit_table_dmas(gate_ap):
                # 8.6MB of table traffic must hit the HW DMA queues AFTER the
                # x/w transfers. Engine queues run independently, so gate the
                # gpsimd desc-gen on a tile produced by the last x DMA.
                gate_scr = consts.tile([128, 8], bf16, tag="gate")
                nc.gpsimd.tensor_copy(gate_scr, gate_ap)
                for g2t in (0, 3, 1, 2):
                    j = pair_slot[g2t]
                    if g2t in SCH_PAIRS:
                        src, dst = schT_d, sch_sb
                    else:
                        src, dst = expfT_d, expf_sb
                    nc.gpsimd.dma_start(
                        out=dst[:, ds(j * 8448, 8448)],
                        in_=src[:, ds(j * 8448, 8448)],
                    )

            # ---- constants ----
            w_sb = {}
            for name, d in (("Wq", Wq_d), ("Wk", Wk_d), ("Wv", Wv_d), ("Wo", Wo_d)):
                t = consts.tile([128, 2, C], bf16, tag=f"w_{name}", name=f"w_{name}")
                nc.sync.dma_start(out=t, in_=d[:].rearrange("(ch p) n -> p ch n", p=128))
                w_sb[name] = t
            ones_sb = consts.tile([128, 32], bf16, tag="ones")
            nc.vector.memset(ones_sb, 1.0)
            ebias = consts.tile([128, 1], f32, tag="ebias")
            nc.vector.memset(ebias, -S_SHIFT)

            # ---- projections (all bf16; q scale folded into host Wq) ----
            kT_sb = [kqv.tile([128, N], bf16, tag=f"kT{m}", name=f"kT{m}")
                     for m in range(2)]
            qT_sb = [kqv.tile([128, QR], bf16, tag=f"qT{m}", name=f"qT{m}")
                     for m in range(2)]
            v_sb = kqv.tile([128, 16, C], bf16, tag="v")

            xTq_r = xTq[:].rearrange("(ch p) n -> p ch n", p=128)
            for j in range(QR // 512):
                xq = xp.tile([128, 2, 512], bf16, tag="x")
                nc.sync.dma_start(out=xq, in_=xTq_r[:, :, ds(512 * j, 512)])
                for m in range(2):
                    ps = spsum.tile([128, 1024], f32, tag="s")
                    for ch in range(2):
                        nc.tensor.matmul(
                            ps[:, :512],
                            lhsT=w_sb["Wq"][:, ch, ts(m, 128)],
                            rhs=xq[:, ch, :],
                            start=(ch == 0),
                            stop=(ch == 1),
                        )
                    nc.scalar.copy(qT_sb[m][:, ds(512 * j, 512)], ps[:, :512])

            xT_r = xT[:].rearrange("(ch p) n -> p ch n", p=128)
            xc_last = None
            for j in range(N // 512):
                xc = xp.tile([128, 2, 512], bf16, tag="x")
                nc.sync.dma_start(out=xc, in_=xT_r[:, :, ds(512 * j, 512)])
                xc_last = xc
                for m in range(2):
                    ps = spsum.tile([128, 1024], f32, tag="s")
                    for ch in range(2):
                        nc.tensor.matmul(
                            ps[:, :512],
                            lhsT=w_sb["Wk"][:, ch, ts(m, 128)],
                            rhs=xc[:, ch, :],
                            start=(ch == 0),
                            stop=(ch == 1),
                        )
                    nc.scalar.copy(kT_sb[m][:, ds(512 * j, 512)], ps[:, :512])
                for t in range(4):
                    kt = 4 * j + t
                    ps = spsum.tile([128, 1024], f32, tag="s")
                    for ch in range(2):
                        nc.tensor.matmul(
                            ps[:, :C],
                            lhsT=xc[:, ch, ts(t, 128)],
                            rhs=w_sb["Wv"][:, ch, :],
                            start=(ch == 0),
                            stop=(ch == 1),
                        )
                    nc.scalar.copy(v_sb[:, kt, :], ps[:, :C])

            emit_table_dmas(xc_last[:, 0, ds(0, 8)])

            if dbg:
                nc.sync.dma_start(out=dbg_t["qT0"][:], in_=qT_sb[0])
                nc.sync.dma_start(out=dbg_t["qT1"][:], in_=qT_sb[1])
                nc.sync.dma_start(out=dbg_t["kT0"][:], in_=kT_sb[0])
                nc.sync.dma_start(out=dbg_t["kT1"][:], in_=kT_sb[1])

            # ---- main attention loops ----
            oT_tiles = []
            for qc in range(2):
                oT = otp.tile([128, 2, 512], bf16, tag="oT", name=f"oT{qc}")
                oT_tiles.append(oT)
            mul_ctr = 0

            def emit_wo(qc):
                oTw = oT_tiles[qc]
                for s in range(4):
                    fps = spsum.tile([128, 1024], f32, tag="s")
                    for ch in range(2):
                        nc.tensor.matmul(
                            fps[:, :C],
                            lhsT=oTw[:, ch, ts(s, 128)],
                            rhs=w_sb["Wo"][:, ch, :],
                            start=(ch == 0),
                            stop=(ch == 1),
                        )
                    stage = stp.tile([128, C], f32, tag="stage")
                    nc.scalar.copy(stage, fps[:, :C])
                    nc.sync.dma_start(
                        out=out_d[ds(512 * qc + 128 * s, 128), :], in_=stage
                    )

            # SCH blocks interleaved mid-sequence: their DVE-serial work
            # overlaps the neighboring ACT blocks' exp stream. qc=0 blocks
            # all come before qc=1 so Wo(qc=0) overlaps the qc=1 stream.
            block_order = [(0, 0), (3, 0), (1, 0), (2, 0), (0, 1), (3, 1),
                           (1, 1), (2, 1)]
            if SCH_PAIRS != (3,):
                block_order = [(g, qc) for g in range(4) for qc in range(2)]
            for g2, qc in block_order:
                is_sch = g2 in SCH_PAIRS
                slot = pair_slot[g2]
                if True:
                    oT = oT_tiles[qc]
                    po_av = 0 if g2 % 2 == 0 else 64
                    po_rs = 64 - po_av
                    half_idx = g2 // 2
                    acc = apsum.tile([128, 512], f32, tag="acc")
                    e_tiles = {}

                    def emit_av(kt):
                        e_t, e_sch = e_tiles.pop(kt)
                        for k in range(2):
                            h = 2 * g2 + k
                            rhs = e_t[:, ts(k, 512)]
                            if e_sch:
                                rhs = rhs.bitcast(bf16)
                            nc.tensor.matmul(
                                acc[ds(po_av + 32 * k, 32), :],
                                lhsT=v_sb[:, kt, ds(32 * h, 32)],
                                rhs=rhs,
                                start=(kt == 0),
                                stop=(kt == 15),
                                tile_position=(0, po_av + 32 * k),
                                skip_group_check=True,
                            )
                            nc.tensor.matmul(
                                acc[ds(po_rs + 32 * k, 32), :],
                                lhsT=ones_sb,
                                rhs=rhs,
                                start=(kt == 0),
                                stop=(kt == 15),
                                tile_position=(0, po_rs + 32 * k),
                                skip_group_check=True,
                            )

                    for kt in range(16):
                        s_ps = spsum.tile([128, 1024], f32, tag="s")
                        if WARM_MM and kt % 3 == 0:
                            # full-array matmul to register PE activity with HAM
                            nc.tensor.matmul(
                                s_ps[:, :64],
                                lhsT=w_sb["Wo"][:, 0, ts(0, 128)],
                                rhs=w_sb["Wo"][:, 0, :64],
                                start=True,
                                stop=True,
                                skip_group_check=True,
                            )
                        for k in range(2):
                            h = 2 * g2 + k
                            i = h % 4
                            nc.tensor.matmul(
                                s_ps[:, ts(k, 512)],
                                lhsT=kT_sb[half_idx][ds(32 * i, 32), ts(kt, 128)],
                                rhs=qT_sb[half_idx][ds(32 * i, 32), ts(qc, 512)],
                                start=True,
                                stop=True,
                                tile_position=(32 * i, 0),
                            )
                        rdw0 = 2 * qc - (kt // 2) + 7
                        woff = 128 if kt % 2 == 0 else 0
                        if is_sch:
                            e_sb = ep.tile([128, 1024], i16, tag="e")
                            e4 = e_sb.rearrange("p (k jj f) -> p k jj f", k=2, jj=2)
                            s4 = s_ps.rearrange("p (k jj f) -> p k jj f", k=2, jj=2)
                            bt4 = sch_view[
                                :, ds(2 * slot, 2), ds(rdw0, 2), ds(woff, 256)
                            ]
                            nc.vector.tensor_add(e4, s4, bt4)
                            if dbg and g2 == SCH_PAIRS[0] and qc == 0 and kt == 0:
                                nc.sync.dma_start(out=dbg_t["e_sch"][:], in_=e_sb)
                            e_tiles[kt] = (e_sb, True)
                        else:
                            e_sb = ep.tile([128, 1024], bf16, tag="e")
                            nc.scalar.activation(e_sb, s_ps, EXP, bias=ebias[:, :])
                            ev = e_sb.rearrange("p (k jj f) -> p k jj f", k=2, jj=2)
                            fv = expf_view[
                                :, ds(2 * slot, 2), ds(rdw0, 2), ds(woff, 256)
                            ]
                            eng = (
                                nc.gpsimd
                                if (GPS_EVERY and mul_ctr % GPS_EVERY == GPS_EVERY - 1)
                                else nc.vector
                            )
                            mul_ctr += 1
                            if dbg and g2 == 0 and qc == 0 and kt == 0:
                                dstage = stp.tile([128, 1024], f32, tag="dst")
                                nc.vector.tensor_copy(dstage, s_ps)
                                nc.sync.dma_start(out=dbg_t["s_act"][:], in_=dstage)
                            eng.tensor_mul(ev, ev, fv)
                            if dbg and g2 == 0 and qc == 0 and kt == 0:
                                nc.scalar.dma_start(out=dbg_t["e_act"][:], in_=e_sb)
                            e_tiles[kt] = (e_sb, False)
                        if kt in (5, 8, 11, 14):
                            for k2 in range(kt - 5, kt - 2):
                                emit_av(k2)
                    for k2 in (12, 13, 14, 15):
                        emit_av(k2)
                    if dbg and g2 == 0 and qc == 0:
                        astage = stp.tile([128, 512], f32, tag="ast")
                        nc.vector.tensor_copy(astage, acc)
                        nc.sync.dma_start(out=dbg_t["acc0"][:], in_=astage)
                    # epilogue: normalize 2 heads into oT.
                    # reciprocal_approx_fast must run at partition base 0 on
                    # HW (base-64 invocations corrupt results; sim is fine).
                    recip = rp.tile([128, 512], f32, tag="recip")
                    rep = rp.tile([128, 512], f32, tag="rep")
                    if po_rs == 0:
                        nc.vector.reciprocal_approx_fast(
                            recip[ds(0, 64), :], acc[ds(0, 64), :]
                        )
                    else:
                        nc.vector.tensor_copy(
                            rep[ds(64, 64), :], acc[ds(64, 64), :]
                        )
                        nc.sync.dma_start(
                            out=rep[ds(0, 64), :], in_=rep[ds(64, 64), :]
                        )
                        nc.vector.reciprocal_approx_fast(
                            recip[ds(0, 64), :], rep[ds(0, 64), :]
                        )
                    if po_av == 0:
                        nc.vector.tensor_mul(
                            oT[ds(0, 64), half_idx, :],
                            acc[ds(0, 64), :],
                            recip[ds(0, 64), :],
                        )
                    else:
                        nc.sync.dma_start(
                            out=rep[ds(64, 64), :], in_=recip[ds(0, 64), :]
                        )
                        nc.vector.tensor_mul(
                            oT[ds(64, 64), half_idx, :],
                            acc[ds(64, 64), :],
                            rep[ds(64, 64), :],
                        )
                if (g2, qc) == (2, 0):
                    emit_wo(0)
            emit_wo(1)

    nc.compile()
    return nc


def _host_tables(T):
    """Per-row-half bias tables in the final SBUF gather layout.

    Returns {r: (expf bf16 [128, nA*8448], sch int16 [128, nS*8448])}.
    Layout: partition p = 16*h2p + w2, free = (pair-slot-local head, rdw 11,
    f 384) where f = 16*drh + w1, gathered value
    G[p,h,rdw,f] = bias_table[h, 4r+rdw, (7-h2p)+drh, 15+w1-w2].
    """
    import ml_dtypes

    bf = ml_dtypes.bfloat16
    T = np.asarray(T, dtype=np.float32)
    p = np.arange(128)
    h2p, w2 = p // 16, p % 16
    f = np.arange(384)
    drh, w1 = f // 16, f % 16
    rh = (7 - h2p)[:, None] + drh[None, :]          # [128, 384]
    rw = 15 + w1[None, :] - w2[:, None]             # [128, 384]
    out = {}
    for r in (0, 1):
        Twin = T[:, 4 * r:4 * r + 11]               # [8, 11, 31, 31]
        G = Twin[:, :, rh, rw]                      # [8, 11, 128, 384]
        G = np.ascontiguousarray(G.transpose(2, 0, 1, 3))  # [128, 8, 11, 384]
        expf = None
        sch = None
        if ACT_PAIRS:
            heads = []
            for g in ACT_PAIRS:
                heads += [2 * g, 2 * g + 1]
            expf = np.ascontiguousarray(
                np.exp(G[:, heads]).reshape(128, -1).astype(bf)
            )
        if SCH_PAIRS:
            heads = []
            for g in SCH_PAIRS:
                heads += [2 * g, 2 * g + 1]
            sch = np.ascontiguousarray(
                np.round(A16 * (G[:, heads] - S_SHIFT) + B16)
                .reshape(128, -1).astype(np.int16)
            )
        out[r] = (expf, sch)
    return out


def _host_inputs(x, Wq, Wk, Wv, Wo, bias_table):
    """Build the 8 per-core input maps."""
    import ml_dtypes

    bf = ml_dtypes.bfloat16
    x = np.asarray(x, dtype=np.float32)
    xf = np.ascontiguousarray(x.reshape(B, N, C))
    qsc = 1.0 / math.sqrt(HD)
    scale = np.full(NH, qsc, np.float32)
    for g in SCH_PAIRS:
        scale[2 * g] = qsc * A16
        scale[2 * g + 1] = qsc * A16
    Wq_s = np.asarray(Wq, np.float32).reshape(C, NH, HD) * scale[None, :, None]
    Ws = {
        "Wq": np.ascontiguousarray(Wq_s.reshape(C, C).astype(bf)),
        "Wk": np.ascontiguousarray(np.asarray(Wk, np.float32).astype(bf)),
        "Wv": np.ascontiguousarray(np.asarray(Wv, np.float32).astype(bf)),
        "Wo": np.ascontiguousarray(np.asarray(Wo, np.float32).astype(bf)),
    }
    tables = _host_tables(bias_table)
    in_maps = []
    for c in range(8):
        b, r = c // 2, c % 2
        expf, sch = tables[r]
        m = {
            "xT": np.ascontiguousarray(xf[b].T.astype(bf)),
            "xTq": np.ascontiguousarray(xf[b, QR * r:QR * (r + 1)].T.astype(bf)),
            **Ws,
        }
        if expf is not None:
            m["expfT"] = expf
        if sch is not None:
            m["schT"] = sch
        in_maps.append(m)
    return in_maps


def kernel(x, Wq, Wk, Wv, Wo, bias_table, _results_hook=None):
    global _NC
    if _NC is None:
        _NC = _build_nc()
    from concourse.bass_utils import run_bass_kernel_spmd

    in_maps = _host_inputs(x, Wq, Wk, Wv, Wo, bias_table)
    res = run_bass_kernel_spmd(_NC, in_maps, core_ids=list(range(8)))
    if _results_hook is not None:
        _results_hook(res)
    out = np.zeros((B, N, C), dtype=np.float32)
    for c in range(8):
        b, r = c // 2, c % 2
        out[b, QR * r:QR * (r + 1)] = res.results[c]["out"]
    D, H, W = 8, 16, 16
    return out.reshape(B, D, H, W, C)
